# revision 32
# baseline (speedup 1.0000x reference)
"""AInnoFace loss kernel for 8 TRN2 NeuronCores.

Strategy: shard the anchor axis (120000 -> pad 120832 = 8*15104) across 8
cores; replicate ground_truth (tiny). Each core computes, for its 15104
anchors x (4 batches * 64 gt):
  - pairwise lnu = ln(inter) - ln(sa+sg), a monotone transform of IoU
    (iou = u/(1-u) with u = inter/(sa+sg); thresholds iou>=0.5 <=> u>=1/3,
    iou<0.4 <=> u<2/7; same argmax). Log-space avoids the expensive DVE
    reciprocal; Ln/exp run on the otherwise-idle ScalarE.
  - per (anchor,batch): max lnu, one-hot argmax (bf16), selected gt box via
    TensorE one-hot matmul (bf16 with hi/lo split of gt coords -> exact),
    elementwise IoU of proposal vs selected box, focal terms, partial sums.
Each core outputs 12 partials (stc_sum[4], str_sum'[4], pos_count[4]);
host sums across cores and applies the final normalization.
"""

import math

import numpy as np

P = 128          # partitions
NT = 118         # tiles per core (anchor columns per partition)
AC = P * NT      # anchors per core = 15104
NCORES = 8
APAD = AC * NCORES  # 120832
A = 120000
B = 4
K = 64
G = B * K        # 256 gt columns

LN13 = math.log(1.0 / 3.0)   # pos threshold in ln(u) space
LN27 = math.log(2.0 / 7.0)   # neg threshold in ln(u) space

_CACHE = {}


def _build_nc():
    from contextlib import ExitStack

    import concourse.bass as bass
    import concourse.mybir as mybir
    from concourse import bass_isa  # noqa: F401

    dt = mybir.dt
    Alu = mybir.AluOpType
    Act = mybir.ActivationFunctionType
    f32 = dt.float32
    bf16 = dt.bfloat16
    f16 = dt.float16

    nc = bass.Bass()

    ssp_h = nc.declare_dram_parameter("ssp", [B, AC, 6], f32, isOutput=False)
    anc_h = nc.declare_dram_parameter("anc", [AC, 4], f32, isOutput=False)
    gt_h = nc.declare_dram_parameter("gt", [B, K, 4], f32, isOutput=False)
    gtblk_h = nc.declare_dram_parameter("gtblk", [P, 32], bf16, isOutput=False)
    gtB_h = nc.declare_dram_parameter("gtB", [P, 1280], f16, isOutput=False)
    out_h = nc.declare_dram_parameter("out", [12, 1], f32, isOutput=True)


    with ExitStack() as stack:
        def sb(name, shape, d=f32):
            return stack.enter_context(nc.sbuf_tensor(name, shape, d))

        def ps(name, shape, d=f32):
            return stack.enter_context(nc.psum_tensor(name, shape, d))

        def sem(name):
            return stack.enter_context(nc.semaphore(name))

        # inputs
        ssp_sb = sb("ssp_sb", [P, B * NT * 6])      # (p, b, c, j)
        anc_sb = sb("anc_sb", [P, NT * 4])          # (p, c, j)
        gtB_sb = sb("gtB_sb", [P, 1280], f16)
        ident_sb = sb("ident_sb", [P, 128])
        onescol_sb = sb("onescol_sb", [P, 1])
        bias001_sb = sb("bias001_sb", [P, 1])
        # block-diagonal gt coords for tb matmuls (bf16 hi/lo, exact sum)
        gtblk_sb = sb("gtblk_sb", [P, 32], bf16)    # per half h: [16h:16h+8]=hi, [16h+8:16h+16]=lo
        # per-anchor derived
        ax2_sb = sb("ax2_sb", [P, NT])
        ay2_sb = sb("ay2_sb", [P, NT])
        sa_sb = sb("sa_sb", [P, NT])
        nax_sb = sb("nax_sb", [P, NT])
        nay_sb = sb("nay_sb", [P, NT])
        # pairwise scratch
        s1x_sb = sb("s1x_sb", [P, G], f16)
        s1y_sb = sb("s1y_sb", [P, G], f16)
        w_sb = sb("w_sb", [P, G], f16)
        h_sb = sb("h_sb", [P, G], f16)
        wr_sb = sb("wr_sb", [P, G], f16)
        hr_sb = sb("hr_sb", [P, G], f16)
        intden_sb = sb("intden_sb", [P, 6 * G], f16)  # x3: [inter(256) | den(256)]
        lnu_sb = sb("lnu_sb", [P, G], f16)
        r1x_sb = sb("r1x_sb", [P, 3 * G], f16)      # ACT-produced, triple buf
        r1y_sb = sb("r1y_sb", [P, 3 * G], f16)
        lnid_sb = sb("lnid_sb", [P, 6 * G], f16)    # x3: [ln(inter) | ln(den)]
        oh_sb = sb("oh_sb", [P, 2 * G])             # one-hot, dbl buf
        ohT_sb = sb("ohT_sb", [P, 2 * G], bf16)     # transposed, dbl buf
        # per-anchor accumulators
        M_sb = sb("M_sb", [P, B * NT], f16)         # max lnu, (p, b, c)
        TB_sb = sb("TB_sb", [P, B * NT * 4])        # (p, b, c, j) xywh
        # final phase scratch (128 x 472)
        px2_sb = sb("px2_sb", [P, B * NT])
        py2_sb = sb("py2_sb", [P, B * NT])
        pa_sb = sb("pa_sb", [P, B * NT])
        tx2_sb = sb("tx2_sb", [P, B * NT])
        ty2_sb = sb("ty2_sb", [P, B * NT])
        ta_sb = sb("ta_sb", [P, B * NT])
        e1_sb = sb("e1_sb", [P, B * NT])
        e2_sb = sb("e2_sb", [P, B * NT])
        e3_sb = sb("e3_sb", [P, B * NT])
        e4_sb = sb("e4_sb", [P, B * NT])
        eiou_sb = sb("eiou_sb", [P, B * NT])
        il_sb = sb("il_sb", [P, B * NT])
        pos_sb = sb("pos_sb", [P, B * NT])
        neg_sb = sb("neg_sb", [P, B * NT])
        p_sb = sb("p_sb", [P, B * NT])
        sp1_sb = sb("sp1_sb", [P, B * NT])
        sp0_sb = sb("sp0_sb", [P, B * NT])
        q2_sb = sb("q2_sb", [P, B * NT])
        p2_sb = sb("p2_sb", [P, B * NT])
        f1_sb = sb("f1_sb", [P, B * NT])
        f0_sb = sb("f0_sb", [P, B * NT])
        sc_sb = sb("sc_sb", [P, B * NT])
        strscr_sb = sb("strscr_sb", [P, B * NT])
        part_sb = sb("part_sb", [P, 12])
        outsb = sb("outsb", [12, 1])
        # psum
        psT0 = ps("psT0", [P, 256], bf16)           # transpose, parity 0
        psT1 = ps("psT1", [P, 256], bf16)           # transpose, parity 1
        tbps0 = ps("tbps0", [P, 16])                # tb matmul out, parity 0
        tbps1 = ps("tbps1", [P, 16])                # tb matmul out, parity 1
        outred = ps("outred", [12, 1])              # final partition reduction
        # semaphores
        s_in = sem("s_in")
        s_inssp = sem("s_inssp")
        s_id = sem("s_id")
        s_prep = sem("s_prep")
        s_bc = sem("s_bc")
        s_gtb = sem("s_gtb")
        s_r1 = sem("s_r1")
        s_int = sem("s_int")
        s_lni = sem("s_lni")
        s_oh = sem("s_oh")
        s_tr = sem("s_tr")
        s_ohT = sem("s_ohT")
        s_mm = sem("s_mm")
        s_tbc = sem("s_tbc")
        s_actf = sem("s_actf")
        s_ei = sem("s_ei")
        s_il = sem("s_il")
        s_part = sem("s_part")
        s_gp = sem("s_gp")
        s_gpc = sem("s_gpc")
        s_out = sem("s_out")
        block = stack.enter_context(nc.Block())

        # views
        ssp6 = ssp_sb[:].rearrange("p (b c j) -> p b c j", b=B, c=NT, j=6)
        anc4 = anc_sb[:].rearrange("p (c j) -> p c j", c=NT, j=4)
        M4 = M_sb[:].rearrange("p (b c) -> p b c", b=B, c=NT)
        TB4 = TB_sb[:].rearrange("p (b c j) -> p b c j", b=B, c=NT, j=4)

        GX1 = gtB_sb[:, 0:256]
        GY1 = gtB_sb[:, 256:512]
        GX2 = gtB_sb[:, 512:768]
        GY2 = gtB_sb[:, 768:1024]
        SG = gtB_sb[:, 1024:1280]

        @block.sync
        def _(sync):
            sync.dma_start(
                anc_sb[:].rearrange("p (c j) -> p c j", c=NT, j=4),
                anc_h[:].rearrange("(p c) j -> p c j", p=P),
            ).then_inc(s_in, 16)
            sync.dma_start(gtblk_sb[:], gtblk_h[:]).then_inc(s_in, 16)
            sync.dma_start(gtB_sb[:], gtB_h[:]).then_inc(s_in, 16)
            sync.dma_start(
                ssp6,
                ssp_h[:].rearrange("b (p c) j -> p b c j", p=P),
            ).then_inc(s_inssp, 16)
            sync.wait_ge(s_gpc, 1)
            sync.dma_start(out_h[:], outsb[:]).then_inc(s_out, 16)

        @block.gpsimd
        def _(gpsimd):
            gpsimd.memset(onescol_sb[:], 1.0)
            gpsimd.memset(bias001_sb[:], 0.01)
            gpsimd.memset(ident_sb[:], 0.0)
            gpsimd.affine_select(
                out=ident_sb[:],
                in_=ident_sb[:],
                compare_op=Alu.not_equal,
                fill=1.0,
                base=0,
                pattern=[[-1, 128]],
                channel_multiplier=1,
            )
            gpsimd.engine_nop().then_inc(s_id, 1)

        @block.vector
        def _(vector):
            vector.wait_ge(s_in, 48)
            vector.engine_nop().then_inc(s_prep, 1)
            # anchor xyxy + area + negated mins
            vector.tensor_tensor(ax2_sb[:], anc4[:, :, 0], anc4[:, :, 2], Alu.add)
            vector.tensor_tensor(ay2_sb[:], anc4[:, :, 1], anc4[:, :, 3], Alu.add)
            vector.tensor_tensor(sa_sb[:], anc4[:, :, 2], anc4[:, :, 3], Alu.mult)
            vector.tensor_scalar(nax_sb[:], anc4[:, :, 0], -1.0, None, Alu.mult)
            vector.tensor_scalar(
                nay_sb[:], anc4[:, :, 1], -1.0, None, Alu.mult
            ).then_inc(s_prep, 1)

            # ---- software-pipelined tile loop ----
            for c in range(NT + 3):
                if c < NT:
                    o3 = (c % 3) * G
                    ax1 = anc4[:, c, 0:1]
                    ay1 = anc4[:, c, 1:2]
                    ax2 = ax2_sb[:, c:c + 1]
                    ay2 = ay2_sb[:, c:c + 1]
                    vector.wait_ge(s_r1, c + 1)
                    vector.tensor_scalar(s1x_sb[:], GX2, ax2, ax1, Alu.min, Alu.subtract)
                    vector.tensor_scalar(s1y_sb[:], GY2, ay2, ay1, Alu.min, Alu.subtract)
                    vector.tensor_tensor(w_sb[:], s1x_sb[:], r1x_sb[:, o3:o3 + G], Alu.subtract)
                    vector.tensor_tensor(h_sb[:], s1y_sb[:], r1y_sb[:, o3:o3 + G], Alu.subtract)
                    vector.tensor_scalar(wr_sb[:], w_sb[:], 0.0, None, Alu.max)
                    o2 = (c % 3) * 2 * G
                    sa = sa_sb[:, c:c + 1]
                    vector.tensor_scalar(
                        intden_sb[:, o2 + G:o2 + 2 * G], SG, sa, None, Alu.add)
                    # inter = relu(w)*h: negative values yield Ln=NaN which the
                    # reduce-max ignores and is_ge compares false -> harmless
                    vector.tensor_tensor(
                        intden_sb[:, o2:o2 + G], wr_sb[:], h_sb[:], Alu.mult
                    ).then_inc(s_int, 1)
                if 2 <= c <= NT + 1:
                    t = c - 2
                    t2 = t % 2
                    o2 = (t % 3) * 2 * G
                    vector.wait_ge(s_lni, t + 1)
                    vector.tensor_tensor(
                        lnu_sb[:], lnid_sb[:, o2:o2 + G],
                        lnid_sb[:, o2 + G:o2 + 2 * G], Alu.subtract)
                    lnu3 = lnu_sb[:].rearrange("p (b k) -> p b k", b=B, k=K)
                    vector.tensor_reduce(
                        M4[:, :, t:t + 1], lnu3, axis=mybir.AxisListType.X, op=Alu.max)
                    if t >= 2:
                        vector.wait_ge(s_tr, t - 1)  # oh[t%2] consumed by PE
                    mbc = M4[:, :, t:t + 1].to_broadcast((P, B, K))
                    vector.tensor_tensor(
                        oh_sb[:, t2 * G:(t2 + 1) * G].rearrange("p (b k) -> p b k", b=B, k=K),
                        lnu3, mbc, Alu.is_ge,
                    ).then_inc(s_oh, 1)

            # ---- final per-anchor phase ----
            vector.wait_ge(s_inssp, 16)
            vector.tensor_tensor(px2_sb[:], ssp6[:, :, :, 0], ssp6[:, :, :, 2], Alu.add)
            vector.tensor_tensor(py2_sb[:], ssp6[:, :, :, 1], ssp6[:, :, :, 3], Alu.add)
            vector.tensor_tensor(pa_sb[:], ssp6[:, :, :, 2], ssp6[:, :, :, 3], Alu.mult)
            # pos/neg masks + counts (ln-space thresholds)
            vector.tensor_scalar(pos_sb[:], M_sb[:], LN13, None, Alu.is_ge)
            vector.tensor_scalar(neg_sb[:], M_sb[:], LN27, None, Alu.is_lt)
            pos4 = pos_sb[:].rearrange("p (b c) -> p b c", b=B, c=NT)
            vector.tensor_reduce(
                part_sb[:, 8:12], pos4, axis=mybir.AxisListType.X, op=Alu.add)
            # focal (ACT produced sp1, sp0, q2, p2)
            vector.wait_ge(s_actf, 1)
            vector.tensor_tensor(f1_sb[:], sp1_sb[:], q2_sb[:], Alu.mult)
            vector.tensor_tensor(f0_sb[:], sp0_sb[:], p2_sb[:], Alu.mult)
            vector.tensor_tensor(f1_sb[:], f1_sb[:], pos_sb[:], Alu.mult)
            vector.tensor_tensor(f0_sb[:], f0_sb[:], neg_sb[:], Alu.mult)
            vector.tensor_scalar(f1_sb[:], f1_sb[:], 0.25, None, Alu.mult)
            vector.tensor_scalar(f0_sb[:], f0_sb[:], 0.75, None, Alu.mult)
            vector.tensor_tensor(sc_sb[:], f1_sb[:], f0_sb[:], Alu.add)
            sc4 = sc_sb[:].rearrange("p (b c) -> p b c", b=B, c=NT)
            vector.tensor_reduce(
                part_sb[:, 0:4], sc4, axis=mybir.AxisListType.X, op=Alu.add)

            # elementwise IoU of proposal vs selected target box
            vector.wait_ge(s_tbc, NT)
            vector.tensor_tensor(tx2_sb[:], TB4[:, :, :, 0], TB4[:, :, :, 2], Alu.add)
            vector.tensor_tensor(ty2_sb[:], TB4[:, :, :, 1], TB4[:, :, :, 3], Alu.add)
            vector.tensor_tensor(ta_sb[:], TB4[:, :, :, 2], TB4[:, :, :, 3], Alu.mult)
            vector.tensor_tensor(e1_sb[:], ssp6[:, :, :, 0], TB4[:, :, :, 0], Alu.max)
            vector.tensor_tensor(e2_sb[:], ssp6[:, :, :, 1], TB4[:, :, :, 1], Alu.max)
            vector.tensor_tensor(e3_sb[:], px2_sb[:], tx2_sb[:], Alu.min)
            vector.tensor_tensor(e4_sb[:], py2_sb[:], ty2_sb[:], Alu.min)
            vector.tensor_tensor(e3_sb[:], e3_sb[:], e1_sb[:], Alu.subtract)  # ew
            vector.tensor_tensor(e4_sb[:], e4_sb[:], e2_sb[:], Alu.subtract)  # eh
            vector.tensor_scalar(e3_sb[:], e3_sb[:], 0.0, None, Alu.max)
            vector.tensor_scalar(e4_sb[:], e4_sb[:], 0.0, None, Alu.max)
            vector.tensor_tensor(e1_sb[:], e3_sb[:], e4_sb[:], Alu.mult)      # einter
            vector.tensor_tensor(e2_sb[:], pa_sb[:], ta_sb[:], Alu.add)
            vector.tensor_tensor(e2_sb[:], e2_sb[:], e1_sb[:], Alu.subtract)  # eden
            vector.reciprocal(e2_sb[:], e2_sb[:])
            vector.tensor_tensor(
                eiou_sb[:], e1_sb[:], e2_sb[:], Alu.mult
            ).then_inc(s_ei, 1)

            vector.wait_ge(s_il, 1)
            vector.tensor_tensor(strscr_sb[:], il_sb[:], pos_sb[:], Alu.mult)
            str4 = strscr_sb[:].rearrange("p (b c) -> p b c", b=B, c=NT)
            vector.tensor_reduce(
                part_sb[:, 4:8], str4, axis=mybir.AxisListType.X, op=Alu.add
            ).then_inc(s_part, 1)

        @block.scalar
        def _(scalar):
            scalar.wait_ge(s_id, 1)
            scalar.wait_ge(s_in, 48)
            scalar.wait_ge(s_prep, 2)  # nax/nay ready for r1 biases
            # ---- pipelined tile loop ----
            for c in range(NT + 4):
                if c < NT:
                    o3 = (c % 3) * G
                    # r1 = relu(g1 - a1) per dim
                    scalar.activation(r1x_sb[:, o3:o3 + G], GX1, Act.Relu,
                                      bias=nax_sb[:, c:c + 1])
                    scalar.activation(r1y_sb[:, o3:o3 + G], GY1, Act.Relu,
                                      bias=nay_sb[:, c:c + 1]).then_inc(s_r1, 1)
                if 1 <= c <= NT:
                    t = c - 1
                    o2 = (t % 3) * 2 * G
                    scalar.wait_ge(s_int, c)
                    scalar.activation(
                        lnid_sb[:, o2:o2 + 2 * G],
                        intden_sb[:, o2:o2 + 2 * G], Act.Ln,
                    ).then_inc(s_lni, 1)
                if 2 <= c <= NT + 1:
                    t = c - 2
                    o = (t % 2) * G
                    ps_t = psT1 if (t % 2) else psT0
                    scalar.wait_ge(s_tr, t + 1)
                    if t >= 2:
                        scalar.wait_ge(s_mm, t - 1)  # ohT[t%2] read by PE mm(t-2)
                    scalar.copy(ohT_sb[:, o:o + G], ps_t[:]).then_inc(s_ohT, 1)
                if 4 <= c:
                    t = c - 4
                    tb_ps = tbps1 if (t % 2) else tbps0
                    scalar.wait_ge(s_mm, t + 1)
                    scalar.copy(
                        TB4[:, :, t, :],
                        tb_ps[:].rearrange("p (b j) -> p b j", b=B, j=4),
                    ).then_inc(s_tbc, 1)
            # focal transcendentals, all in ln/exp LUT set:
            # sp1 = softplus(-L) = -log sigmoid(L); sp0 = softplus(L)
            # p^2 = exp(-2*sp1); (1-p)^2 = exp(-2*sp0)
            L = ssp6[:, :, :, 4]
            scalar.wait_ge(s_inssp, 16)
            scalar.activation(p_sb[:], L, Act.Exp, scale=-1.0)
            scalar.activation(sp1_sb[:], p_sb[:], Act.Ln, bias=1.0)
            scalar.activation(p2_sb[:], L, Act.Exp)
            scalar.activation(sp0_sb[:], p2_sb[:], Act.Ln, bias=1.0)
            scalar.activation(q2_sb[:], sp0_sb[:], Act.Exp, scale=-2.0)
            scalar.activation(p2_sb[:], sp1_sb[:], Act.Exp, scale=-2.0).then_inc(s_actf, 1)
            # il = ln(eiou + 0.01)
            scalar.wait_ge(s_ei, 1)
            scalar.activation(il_sb[:], eiou_sb[:], Act.Ln, bias=bias001_sb[:]).then_inc(s_il, 1)
            scalar.wait_ge(s_gp, 1)
            scalar.copy(outsb[:], outred[0:12, 0:1]).then_inc(s_gpc, 1)

        @block.tensor
        def _(tensor):
            tensor.wait_ge(s_id, 1)
            for c in range(1, NT + 3):
                if c <= NT:
                    t = c - 1
                    o = (t % 2) * G
                    ps_t = psT1 if (t % 2) else psT0
                    tensor.wait_ge(s_oh, c)
                    if t >= 2:
                        tensor.wait_ge(s_ohT, t - 1)  # psT[t%2] copied out
                    tensor.transpose(ps_t[:, 0:128], oh_sb[:, o:o + 128], ident_sb[:])
                    tensor.transpose(
                        ps_t[:, 128:256], oh_sb[:, o + 128:o + 256], ident_sb[:]
                    ).then_inc(s_tr, 1)
                if c >= 3:
                    t = c - 3
                    t2 = t % 2
                    tb_ps = tbps1 if t2 else tbps0
                    tensor.wait_ge(s_ohT, t + 1)
                    if t >= 2:
                        tensor.wait_ge(s_tbc, t - 1)  # tbps[t%2] consumed
                    # per half: hi and lo matmuls accumulate into the same
                    # psum region -> tb = hi + lo exactly (f32 accumulate)
                    for half in range(2):
                        lhs = ohT_sb[:, t2 * G + 128 * half:t2 * G + 128 * half + 128]
                        last = tensor.matmul(
                            tb_ps[:, 8 * half:8 * half + 8],
                            lhs, gtblk_sb[:, 16 * half:16 * half + 8],
                            start=True, stop=False)
                        last = tensor.matmul(
                            tb_ps[:, 8 * half:8 * half + 8],
                            lhs, gtblk_sb[:, 16 * half + 8:16 * half + 16],
                            start=False, stop=True)
                    last.then_inc(s_mm, 1)
            tensor.wait_ge(s_part, 1)
            tensor.matmul(outred[:], part_sb[:], onescol_sb[:],
                          start=True, stop=True).then_inc(s_gp, 1)

    nc.freeze()
    return nc


def _make_gtB(gt):
    """(128, 1280) fp16 broadcast tiles [gx1|gy1|gx2|gy2|sg], col = b*64+k."""
    g = gt.astype(np.float32)
    x1 = g[..., 0]; y1 = g[..., 1]
    x2 = g[..., 0] + g[..., 2]; y2 = g[..., 1] + g[..., 3]
    sg = g[..., 2] * g[..., 3]
    row = np.concatenate([x1.reshape(-1), y1.reshape(-1), x2.reshape(-1),
                          y2.reshape(-1), sg.reshape(-1)])
    return np.broadcast_to(row, (P, 1280)).astype(np.float16)


def _make_gtblk(gt):
    """(128, 32) bf16 block-diagonal [hi | lo] gt coords for tb matmuls.
    half h: rows 0:64 = batch 2h, rows 64:128 = batch 2h+1;
    cols 16h+4r : +4 = hi(batch 2h+r), cols 16h+8+4r : +4 = lo."""
    import ml_dtypes
    g = np.zeros((P, 32), np.float32)
    for half in range(2):
        for r in range(2):
            b = 2 * half + r
            rows = slice(64 * r, 64 * r + 64)
            hi = gt[b].astype(np.float32)
            hib = ((hi.view(np.uint32) + 0x8000) & 0xFFFF0000).view(np.float32)
            g[rows, 16 * half + 4 * r:16 * half + 4 * r + 4] = hib
            g[rows, 16 * half + 8 + 4 * r:16 * half + 8 + 4 * r + 4] = hi - hib
    return g.astype(ml_dtypes.bfloat16)


def _prepare_shards(ss_proposal, anchors, ground_truth):
    ssp = np.ascontiguousarray(np.asarray(ss_proposal, dtype=np.float32))
    anc = np.ascontiguousarray(np.asarray(anchors, dtype=np.float32))
    gt = np.ascontiguousarray(np.asarray(ground_truth, dtype=np.float32))
    npad = APAD - A
    # pad anchors far away ([50,50,1,1]); pad logits -30 (focal contributes 0,
    # exp(30) stays finite)
    anc_pad = np.concatenate(
        [anc, np.tile(np.array([50.0, 50.0, 1.0, 1.0], np.float32), (npad, 1))], axis=0)
    ssp_padrow = np.zeros((B, npad, 6), np.float32)
    ssp_padrow[..., :4] = np.array([50.0, 50.0, 1.0, 1.0], np.float32)
    ssp_padrow[..., 4] = -30.0
    ssp_pad = np.concatenate([ssp, ssp_padrow], axis=1)

    gtblk = _make_gtblk(gt)
    gtB = _make_gtB(gt)
    in_maps = []
    for i in range(NCORES):
        sl = slice(i * AC, (i + 1) * AC)
        in_maps.append({
            "ssp": np.ascontiguousarray(ssp_pad[:, sl, :]),
            "anc": np.ascontiguousarray(anc_pad[sl]),
            "gt": gt,
            "gtblk": gtblk,
            "gtB": gtB,
        })
    return in_maps


def _combine(parts):
    # parts: list of (12,) arrays per core; str partials carry a + sign
    # for sum(pos * ln(eiou+0.01)) so negate to get str_sum.
    tot = np.sum([np.asarray(p).reshape(12).astype(np.float64) for p in parts], axis=0)
    stc, strs, cnt = tot[0:4], -tot[4:8], tot[8:12]
    safe = np.where(cnt > 0, cnt, 1.0)
    total = (stc / safe + np.where(cnt > 0, strs / safe, 0.0)).sum() / B
    return np.float32(total)


def kernel(ss_proposal, anchors, ground_truth):
    from concourse.bass_utils import run_bass_kernel_spmd
    if "nc" not in _CACHE:
        _CACHE["nc"] = _build_nc()
    nc = _CACHE["nc"]
    in_maps = _prepare_shards(ss_proposal, anchors, ground_truth)
    res = run_bass_kernel_spmd(nc, in_maps, list(range(NCORES)))
    parts = [res.results[i]["out"] for i in range(NCORES)]
    return np.asarray(_combine(parts), dtype=np.float32)
